# revision 6
# baseline (speedup 1.0000x reference)
"""BiLSTM-CRF forward NLL on 8 Trainium2 NeuronCores.

Sharding: pure data-parallel over batch (8 sequences per core), params
replicated. Per core: embedding gather -> bulk input matmuls -> 2-layer
BiLSTM recurrence (fwd/bwd chains interleaved per layer) -> emissions ->
CRF forward scan in probability space -> partial (em_sel, denom) pair.
Host sums partials with the label-dependent numerator constant.

Key restructurings (validated exactly against the reference in fp32):
  * LSTM cell uses a single tanh activation per step over all 4 gates:
    sigmoid(x) = (tanh(x/2)+1)/2, with the tanh(0.5*...) instruction scale
    folded into host-prescaled weights (g-gate rows x2). The cell tracks
    C = 2c and H = 2h; every consumer of h (recurrent weights, layer-1
    input weights, output projection) is pre-halved on the host.
  * CRF forward scan runs in probability space: P_{t+1} = (E^T P_t) * em
    with E = exp(trans) stationary on the PE, em' = em + b_out - log(L)
    exponentiated in bulk. The per-step shift log(L) cancels exactly
    between numerator and denominator. Periodic renorm keeps P ~ O(1).
"""

import os
import sys

import numpy as np

sys.path.insert(0, "/opt/trn_rl_repo")

import concourse.bass as bass
import concourse.tile as tile
from concourse import bacc, mybir
from concourse.bass_utils import run_bass_kernel_spmd

B, T, V, D, HD, L = 64, 512, 100000, 300, 256, 9
H = 128
NCORES = 8
BL = B // NCORES          # sequences per core
DPAD = 384                # D padded so DMA-transpose chunks are 128 wide
KCH = (128, 128, 128)     # K chunks of DPAD
CBAR = float(np.log(L))   # per-step CRF shift (cancels in num - denom)
RENORM_TS = (170, 340)

f32 = mybir.dt.float32
bf16 = mybir.dt.bfloat16
i32 = mybir.dt.int32
ALU = mybir.AluOpType
ACTF = mybir.ActivationFunctionType


# ---------------------------------------------------------------------------
# device program
# ---------------------------------------------------------------------------

def build_program(Tsteps=T, phases=99):
    NT = Tsteps * BL
    NCK = max(1, NT // 512)            # bulk matmul N chunks
    NCOLS = NT // NCK
    NTILES = NT // 128                 # gather tiles
    assert NT % 128 == 0 and NT % NCK == 0

    nc = bacc.Bacc("TRN2", target_bir_lowering=False, debug=False)

    def din(name, shape, dt):
        return nc.dram_tensor(name, shape, dt, kind="ExternalInput").ap()

    words = din("words", [NTILES, 128, 1], i32)
    emb = din("emb", [V, DPAD], bf16)
    ident = din("ident", [128, 128], bf16)
    # lhsT weights, gate-major free dim (slots i,f,g,o each 128 wide)
    wih0 = din("wih0", [2, 3, 128, 512], bf16)     # [dir][kchunk][K][4*128]
    wih1 = din("wih1", [2, 2, 128, 512], bf16)     # [dir][h0-dir kchunk][K][4*128]
    whh = din("whh", [2, 2, 128, 512], bf16)       # [layer][dir][K=H][4*128]
    biases = din("biases", [2, 2, 128, 4], f32)    # [layer][dir][hidden][gate]
    woutT = din("woutT", [2, 128, L], bf16)        # [h1-dir kchunk][K][L]
    bout = din("bout", [L, 1], f32)                # b_out - CBAR
    startv = din("startv", [L, 1], f32)
    expE = din("expE", [L, L], f32)                # exp(trans)
    expend = din("expend", [L, 1], f32)            # exp(end_t)
    oh = din("oh", [L, NT], bf16)                  # label one-hot, (t,b) order
    res = nc.dram_tensor("res", [1, 2], f32, kind="ExternalOutput").ap()

    with tile.TileContext(nc) as tc:
        _emit(tc, nc, Tsteps, NT, NCK, NCOLS, NTILES,
              words, emb, ident, wih0, wih1, whh, biases, woutT, bout,
              startv, expE, expend, oh, res, phases)
    nc.compile()
    return nc


def _emit(tc, nc, Tsteps, NT, NCK, NCOLS, NTILES,
          words, emb, ident, wih0, wih1, whh, biases, woutT, bout,
          startv, expE, expend, oh, res, phases=99):
    from contextlib import ExitStack

    ctx = ExitStack()
    with ctx:
        consts = ctx.enter_context(tc.tile_pool(name="consts", bufs=1))
        states = ctx.enter_context(tc.tile_pool(name="states", bufs=1))

        # ---- persistent SBUF tiles ----
        ident_sb = consts.tile([128, 128], bf16, tag="ident")
        nc.sync.dma_start(ident_sb[:], ident[:])
        whh_sb = {}
        for l in range(2):
            for d in range(2):
                t_ = consts.tile([128, 512], bf16, name=f"whh{l}{d}")
                nc.sync.dma_start(t_[:], whh[l, d])
                whh_sb[l, d] = t_
        bias_sb = {}
        for l in range(2):
            for d in range(2):
                t_ = consts.tile([128, 4], f32, name=f"bias{l}{d}")
                nc.sync.dma_start(t_[:], biases[l, d])
                bias_sb[l, d] = t_
        wih0_sb = {}
        for d in range(2):
            for c in range(3):
                t_ = consts.tile([128, 512], bf16, name=f"wih0_{d}{c}")
                nc.sync.dma_start(t_[:], wih0[d, c])
                wih0_sb[d, c] = t_
        wih1_sb = {}
        for d in range(2):
            for k in range(2):
                t_ = consts.tile([128, 512], bf16, name=f"wih1_{d}{k}")
                nc.sync.dma_start(t_[:], wih1[d, k])
                wih1_sb[d, k] = t_

        h_hist = {}
        for l in range(2):
            for d in range(2):
                h_hist[l, d] = states.tile([128, NT], bf16, name=f"h{l}{d}")
        # persistent per-dir cell tile: cols [o i f g | C] (x BL each)
        ycell = [states.tile([128, 5 * BL], f32, name=f"yc{d}") for d in range(2)]
        tcl_st = [states.tile([128, BL], f32, name=f"tcl{d}") for d in range(2)]

        # xp for the two directions of the current layer (reused across layers)
        xp_sb = [states.tile([128, 4 * NT], bf16, name=f"xp{d}") for d in range(2)]

        def bulk_phase(layer, srcs_of_d, pool_name):
            with tc.tile_pool(name=pool_name, bufs=4, space="PSUM") as bp:
                for d in range(2):
                    srcs = srcs_of_d(d)
                    for nck in range(NCK):
                        nsl = slice(nck * NCOLS, (nck + 1) * NCOLS)
                        for slot in range(4):
                            pt = bp.tile([128, NCOLS], f32, tag="pt", name="pt")
                            for ki, (src, wt, kk) in enumerate(srcs):
                                nc.tensor.matmul(
                                    pt[:],
                                    lhsT=wt[:kk, slot * 128:(slot + 1) * 128],
                                    rhs=src[:kk, nsl],
                                    start=(ki == 0), stop=(ki == len(srcs) - 1),
                                )
                            xv = xp_sb[d][:].rearrange(
                                "p (t g b) -> p t g b", g=4, b=BL)
                            tpc = NCOLS // BL
                            nc.vector.tensor_scalar(
                                out=xv[:, nck * tpc:(nck + 1) * tpc, slot, :],
                                in0=pt[:].rearrange("p (t b) -> p t b", b=BL),
                                scalar1=bias_sb[layer, d][:, slot:slot + 1],
                                scalar2=None, op0=ALU.add,
                            )

        def recur_phase(layer):
            # Per-cell emission (all of dir-0's step, then all of dir-1's):
            # each engine's in-order queues then settle into a ~half-period
            # phase shift between the two chains, overlapping their latency.
            for d in range(2):
                nc.vector.memset(ycell[d][:, 4 * BL:5 * BL], 0.0)
            with tc.tile_pool(name=f"gates{layer}", bufs=4, space="PSUM") as gpp, \
                 tc.tile_pool(name=f"w{layer}", bufs=4) as wp:
                for t in range(Tsteps):
                    taus = (t, Tsteps - 1 - t)
                    first = (t == 0)
                    for d in range(2):
                        tau = taus[d]
                        y = ycell[d]
                        gp = gpp.tile([128, 4 * BL], f32, tag=f"gp{d}", name=f"gp{d}")
                        nc.tensor.matmul(gp[:], lhsT=ident_sb[:],
                                         rhs=xp_sb[d][:, tau * 4 * BL:(tau + 1) * 4 * BL],
                                         start=True, stop=first)
                        if not first:
                            prev = tau - 1 if d == 0 else tau + 1
                            hh = h_hist[layer, d]
                            whh_t = whh_sb[layer, d]
                            for slot in range(4):
                                nc.tensor.matmul(
                                    gp[:, slot * BL:(slot + 1) * BL],
                                    lhsT=whh_t[:, slot * 128:(slot + 1) * 128],
                                    rhs=hh[:, prev * BL:(prev + 1) * BL],
                                    start=False, stop=(slot == 3))
                        # y[0:4BL] = tanh(gates/2), slots (o,i,f,g)
                        nc.scalar.activation(y[:, 0:4 * BL], gp[:], ACTF.Tanh,
                                             scale=0.5)
                        # w = [(yi+1)*yg | (yf+1)*C_old] = [v2 | z]
                        w = wp.tile([128, 2 * BL], f32, tag=f"w{d}", name=f"w{d}")
                        nc.vector.scalar_tensor_tensor(
                            w[:], in0=y[:, BL:3 * BL], scalar=1.0,
                            in1=y[:, 3 * BL:5 * BL], op0=ALU.add, op1=ALU.mult)
                        # C = 0.5*z + v2
                        nc.vector.scalar_tensor_tensor(
                            y[:, 4 * BL:5 * BL], in0=w[:, BL:2 * BL], scalar=0.5,
                            in1=w[:, 0:BL], op0=ALU.mult, op1=ALU.add)
                        nc.scalar.activation(tcl_st[d][:], y[:, 4 * BL:5 * BL],
                                             ACTF.Tanh, scale=0.5)
                        nc.vector.scalar_tensor_tensor(
                            h_hist[layer, d][:, tau * BL:(tau + 1) * BL],
                            in0=y[:, 0:BL], scalar=1.0, in1=tcl_st[d][:],
                            op0=ALU.add, op1=ALU.mult)

        # =================================================================
        # Phase 1: embedding gather + transpose, then layer-0 bulk matmul
        # =================================================================
        def bail():
            with tc.tile_pool(name="bail", bufs=1) as bl:
                z = bl.tile([1, 2], f32, name="zbail")
                nc.vector.memset(z[:], 0.0)
                nc.sync.dma_start(res[:], z[:])

        if phases <= 0:
            bail()
            return
        with tc.tile_pool(name="xT", bufs=1) as xTp:
            x_T = [xTp.tile([k, NT], bf16, name=f"xT{c}") for c, k in enumerate(KCH)]
            with tc.tile_pool(name="gath", bufs=4) as gp, \
                 tc.tile_pool(name="idx", bufs=4) as ip:
                for i in range(NTILES):
                    idx = ip.tile([128, 1], i32, tag="idx", name="idx")
                    nc.sync.dma_start(idx[:], words[i])
                    g = gp.tile([128, DPAD], bf16, tag="g", name="g")
                    nc.gpsimd.indirect_dma_start(
                        out=g[:], out_offset=None, in_=emb[:],
                        in_offset=bass.IndirectOffsetOnAxis(ap=idx[:, :1], axis=0),
                    )
                    off = 0
                    for c, k in enumerate(KCH):
                        nc.sync.dma_start_transpose(
                            x_T[c][:, i * 128:(i + 1) * 128], g[:, off:off + k])
                        off += k
            if phases >= 2:
                bulk_phase(0, lambda d: [(x_T[c], wih0_sb[d, c], KCH[c])
                                         for c in range(3)], "bulk0")
        if phases <= 2:
            bail()
            return
        recur_phase(0)
        if phases <= 3:
            bail()
            return
        bulk_phase(1, lambda d: [(h_hist[0, k], wih1_sb[d, k], 128)
                                 for k in range(2)], "bulk1")
        recur_phase(1)
        if phases <= 4:
            bail()
            return

        # =================================================================
        # Phase 4: emissions em' = 0.5*w_out @ H1 + (b_out - cbar)
        # =================================================================
        with tc.tile_pool(name="crf", bufs=1) as crf, \
             tc.tile_pool(name="small", bufs=1) as small:
            em_sb = crf.tile([L, NT], bf16, name="em")
            expem = crf.tile([L, NT], f32, name="expem")
            oh_sb = crf.tile([L, NT], bf16, name="oh_sb")
            scr = crf.tile([L, NT], f32, name="scr")
            bout_sb = small.tile([L, 1], f32, name="bout_sb")
            start_sb = small.tile([L, 1], f32, name="start_sb")
            expE_sb = small.tile([L, L], f32, name="expE_sb")
            expend_sb = small.tile([L, 1], f32, name="expend_sb")
            ones9 = small.tile([L, L], f32, name="ones9")
            woutT_sb = [small.tile([128, L], bf16, name=f"wo{k}") for k in range(2)]
            nc.sync.dma_start(bout_sb[:], bout[:])
            nc.sync.dma_start(start_sb[:], startv[:])
            nc.sync.dma_start(expE_sb[:], expE[:])
            nc.sync.dma_start(expend_sb[:], expend[:])
            nc.sync.dma_start(oh_sb[:], oh[:])
            nc.vector.memset(ones9[:], 1.0)
            for k in range(2):
                nc.sync.dma_start(woutT_sb[k][:], woutT[k])

            if phases <= 5:
                nc.sync.dma_start(res[:], expE_sb[0:1, 0:2])
                return
            with tc.tile_pool(name="emp", bufs=4, space="PSUM") as emp:
                for nck in range(NCK):
                    nsl = slice(nck * NCOLS, (nck + 1) * NCOLS)
                    pt = emp.tile([L, NCOLS], f32, tag="pt", name="pt")
                    for k in range(2):
                        nc.tensor.matmul(pt[:], lhsT=woutT_sb[k][:],
                                         rhs=h_hist[1, k][:, nsl],
                                         start=(k == 0), stop=(k == 1))
                    nc.vector.tensor_scalar(
                        out=em_sb[:, nsl], in0=pt[:],
                        scalar1=bout_sb[:, 0:1], scalar2=None, op0=ALU.add)

            # =============================================================
            # Phase 5: CRF forward scan (prob space) + numerator reduce
            # =============================================================
            if phases <= 6:
                nc.gpsimd.dma_start(res[:], em_sb[0:1, 0:2])
                return
            P = crf.tile([L, BL], f32, name="P")
            lnacc = crf.tile([1, BL], f32, name="lnacc")
            num9 = crf.tile([L, 1], f32, name="num9")
            nc.vector.memset(lnacc[:], 0.0)
            nc.vector.memset(num9[:], 0.0)
            if phases != 70:
                nc.scalar.activation(P[:], em_sb[:, 0:BL], ACTF.Exp,
                                     bias=start_sb[:, 0:1])
            else:
                nc.vector.memset(P[:], 0.1)
            if phases != 71:
                nc.scalar.activation(expem[:, BL:NT], em_sb[:, BL:NT], ACTF.Exp)
            else:
                nc.vector.memset(expem[:], 0.5)
            if phases != 72:
                nc.vector.tensor_tensor(out=scr[:], in0=em_sb[:], in1=oh_sb[:],
                                        op=ALU.mult)
                nc.vector.tensor_reduce(num9[:, 0:1], scr[:],
                                        axis=mybir.AxisListType.X, op=ALU.add)

            if phases <= 7 or 70 <= phases <= 79:
                nc.sync.dma_start(res[:], expem[0:1, 0:2])
                return
            halves = [(0, BL // 2), (BL // 2, BL)]
            with tc.tile_pool(name="crfp", bufs=4, space="PSUM") as cp, \
                 tc.tile_pool(name="crfp2", bufs=1, space="PSUM") as cp2, \
                 tc.tile_pool(name="crfs", bufs=4) as cs:
                for t in range(1, Tsteps if phases > 8 else 3):
                    for (b0, b1) in halves:
                        w = b1 - b0
                        sp = cp.tile([L, w], f32, tag="sp", name="sp")
                        nc.tensor.matmul(sp[:], lhsT=expE_sb[:], rhs=P[:, b0:b1],
                                         start=True, stop=True)
                        nc.vector.tensor_tensor(
                            out=P[:, b0:b1], in0=sp[:],
                            in1=expem[:, t * BL + b0: t * BL + b1], op=ALU.mult)
                    if t in RENORM_TS:
                        for (b0, b1) in halves:
                            w = b1 - b0
                            srow = cp2.tile([L, w], f32, tag="srow", name="srow")
                            nc.tensor.matmul(srow[:], lhsT=ones9[:],
                                             rhs=P[:, b0:b1], start=True, stop=True)
                            lns = cs.tile([1, w], f32, tag="lns", name="lns")
                            nc.scalar.activation(lns[:], srow[0:1, :], ACTF.Ln)
                            nc.vector.tensor_tensor(out=lnacc[:, b0:b1],
                                                    in0=lnacc[:, b0:b1],
                                                    in1=lns[:], op=ALU.add)
                            rec = cs.tile([L, w], f32, tag="rec", name="rec")
                            nc.vector.reciprocal(rec[:], srow[:])
                            nc.vector.tensor_tensor(
                                out=P[:, b0:b1], in0=P[:, b0:b1],
                                in1=rec[:], op=ALU.mult)
                # final: denom_b = ln(expend @ P) + lnacc
                zrow = cp2.tile([1, BL], f32, tag="srow", name="zrow")
                nc.tensor.matmul(zrow[:], lhsT=expend_sb[:, 0:1], rhs=P[:],
                                 start=True, stop=True)
                lnz = cs.tile([1, BL], f32, tag="lns", name="lnz")
                nc.scalar.activation(lnz[:], zrow[:], ACTF.Ln)
                nc.vector.tensor_tensor(out=lnz[:], in0=lnz[:], in1=lnacc[:],
                                        op=ALU.add)
                dsc = cs.tile([1, 1], f32, tag="dsc", name="dsc")
                nc.vector.tensor_reduce(dsc[:], lnz[:], axis=mybir.AxisListType.X,
                                        op=ALU.add)
                npsum = cp2.tile([1, 1], f32, tag="np", name="npsum")
                nc.tensor.matmul(npsum[:], lhsT=ones9[:, 0:1], rhs=num9[:, 0:1],
                                 start=True, stop=True)
                out_sb = cs.tile([1, 2], f32, tag="out_sb", name="out_sb")
                nc.vector.tensor_scalar(out=out_sb[:, 0:1], in0=npsum[:],
                                        scalar1=0.0, scalar2=None, op0=ALU.add)
                nc.vector.tensor_scalar(out=out_sb[:, 1:2], in0=dsc[:],
                                        scalar1=0.0, scalar2=None, op0=ALU.add)
                nc.sync.dma_start(res[:], out_sb[:])


# ---------------------------------------------------------------------------
# host side
# ---------------------------------------------------------------------------

def _prescale(w_ih, w_hh, b_ih, b_hh, h_in_doubled):
    """Gate-slot layout is torch order (i,f,g,o). Returns fp32 arrays."""
    sg = np.ones((4, 1), np.float32)
    sg[2] = 2.0                       # g gate rows x2 (tanh scale 0.5 trick)
    srows = np.repeat(sg, H, axis=0)  # [512, 1]
    wih = w_ih.astype(np.float32) * srows
    whh_ = w_hh.astype(np.float32) * srows * 0.5
    b = (b_ih + b_hh).astype(np.float32) * srows[:, 0]
    if h_in_doubled:
        wih = wih * 0.5
    return wih, whh_, b


GATE_ORDER = (3, 0, 1, 2)   # device slot s holds torch gate GATE_ORDER[s]: o,i,f,g


def _lhsT_gate_major(w, kchunks):
    """w: [4H, K] fp32 -> [nchunk, 128, 512] bf16 lhsT (zero-padded K)."""
    outs = []
    off = 0
    for kk in kchunks:
        blk = np.zeros((128, 512), np.float32)
        take = min(kk, w.shape[1] - off)
        for slot, g in enumerate(GATE_ORDER):
            blk[:take, slot * 128:(slot + 1) * 128] = \
                w[g * H:(g + 1) * H, off:off + take].T
        outs.append(blk)
        off += kk
    return np.stack(outs).astype(np.dtype("bfloat16"))


_PROG_CACHE = {}


def _get_program(Tsteps):
    if Tsteps not in _PROG_CACHE:
        _PROG_CACHE[Tsteps] = build_program(Tsteps)
    return _PROG_CACHE[Tsteps]


def prepare_inputs(inputs, Tsteps=T):
    """Build the per-core input maps + the host numerator constants."""
    bfl = np.dtype("bfloat16")
    words = np.asarray(inputs["word_batch"]).astype(np.int64)
    labels = np.asarray(inputs["label_batch"]).astype(np.int64)
    emb = np.asarray(inputs["emb"], np.float32)
    words = words[:, :Tsteps]
    labels = labels[:, :Tsteps]

    embp = np.zeros((V, DPAD), np.float32)
    embp[:, :D] = emb
    embp = embp.astype(bfl)

    ident = np.eye(128, dtype=np.float32).astype(bfl)

    wih0_l, whh_l, wih1_l, bias_l = [], [], [], []
    for layer, (wihk, whhk, bihk, bhhk) in enumerate(
            [("w_ih_l0", "w_hh_l0", "b_ih_l0", "b_hh_l0"),
             ("w_ih_l1", "w_hh_l1", "b_ih_l1", "b_hh_l1")]):
        for d in range(2):
            wih, whh_, b = _prescale(
                np.asarray(inputs[wihk])[d], np.asarray(inputs[whhk])[d],
                np.asarray(inputs[bihk])[d], np.asarray(inputs[bhhk])[d],
                h_in_doubled=(layer == 1))
            if layer == 0:
                wihp = np.zeros((512, DPAD), np.float32)
                wihp[:, :D] = wih
                wih0_l.append(_lhsT_gate_major(wihp, KCH))
            else:
                wih1_l.append(_lhsT_gate_major(wih, (128, 128)))
            whh_l.append(_lhsT_gate_major(whh_, (128,)))
            bias_l.append(b.reshape(4, H)[list(GATE_ORDER)].T)  # [128, 4]
    wih0 = np.stack(wih0_l)                       # [2, 3, 128, 512]
    wih1 = np.stack(wih1_l)                       # [2, 2, 128, 512]
    whh = np.stack(whh_l).reshape(2, 2, 1, 128, 512)[:, :, 0]
    biases = np.stack(bias_l).reshape(2, 2, 128, 4).astype(np.float32)

    w_out = np.asarray(inputs["w_out"], np.float32) * 0.5   # [L, 2H]
    woutT = np.stack([w_out[:, :H].T, w_out[:, H:].T]).astype(bfl)  # [2,128,L]
    bout = (np.asarray(inputs["b_out"], np.float32) - CBAR).reshape(L, 1)
    startv = np.asarray(inputs["start_t"], np.float32).reshape(L, 1)
    expE = np.exp(np.asarray(inputs["trans"], np.float32))
    expend = np.exp(np.asarray(inputs["end_t"], np.float32)).reshape(L, 1)

    NT = Tsteps * BL
    in_maps = []
    num_consts = []
    start_t = np.asarray(inputs["start_t"], np.float32)
    end_t = np.asarray(inputs["end_t"], np.float32)
    trans = np.asarray(inputs["trans"], np.float32)
    for c in range(NCORES):
        bs = slice(c * BL, (c + 1) * BL)
        wc = words[bs]                        # [BL, Tsteps]
        lc = labels[bs]
        toks = wc.T.reshape(-1).astype(np.int32)          # (t, b) order
        ohc = (lc.T.reshape(1, -1) == np.arange(L).reshape(L, 1))
        in_maps.append({
            "words": toks.reshape(NT // 128, 128, 1),
            "emb": embp, "ident": ident,
            "wih0": wih0, "wih1": wih1, "whh": whh, "biases": biases,
            "woutT": woutT, "bout": bout, "startv": startv,
            "expE": expE, "expend": expend,
            "oh": ohc.astype(bfl),
        })
        num_consts.append(
            float(start_t[lc[:, 0]].sum())
            + float(trans[lc[:, :-1], lc[:, 1:]].sum())
            + float(end_t[lc[:, -1]].sum()))
    return in_maps, num_consts


def kernel(**inputs):
    in_maps, num_consts = prepare_inputs(inputs, T)
    nc = _get_program(T)
    out = run_bass_kernel_spmd(nc, in_maps, list(range(NCORES)))
    llh = 0.0
    for c in range(NCORES):
        r = out.results[c]["res"].reshape(2).astype(np.float64)
        llh += num_consts[c] + r[0] - r[1]
    return np.float32(-llh)


if __name__ == "__main__":
    np.random.seed(0)
    print("building program (small T) ...")
    build_program(16)
    print("ok")

